# revision 34
# baseline (speedup 1.0000x reference)
"""Bass/Tile TRN2 kernel for nn_MultiHead (B=4, T=2048, C=1024, H=16, D=64).

Sharding: 8 cores = batch(4) x head-group(2).  Each core computes, for its
batch b and its 8 heads, the full attention block and a *partial* output
projection (its 512 rows of Wo).  Host sums the two partials per batch and
adds the bias.

On-device layout (v2 — fp8 QK + transposed PV):
  - q/k/v arrive pre-transposed from the host ([C, T]); q/k/Wq/Wk in fp8
    (host pre-scales W by 64 to clear the fp8 subnormal range)
  - q-proj / k-proj: fp8 DoubleRow over c-block pairs -> qh^T (hd, t) PSUM;
    evac to an fp8 staging tile, then 4 small SP-queue DMAs repack each
    (head, d) 64-row block into DoubleRow layout [32p, o2, t] (d = o*32+p)
  - QK^T: fp8 DoubleRow, K=64 per head -> logits^T (tk, tq), both heads of
    a pair in one 2-bank PSUM tile; ONE 3D-AP exp covers both heads
  - softmax: ones-column in vh gives row sums inside the PV matmul; causal
    mask multiplied into P (DVE) on diagonal blocks only
  - PV TRANSPOSED: out [tq1: per 128-row tq block] acc[tq,65] += P^T-block
    (lhsT, [tk,128]) @ vh_aug (rhs, [tk,65]) -- 65-col outputs cut PE time
    2x vs [65,tq] and put S on a per-partition column:
      recip(acc[:,64]) -> [128,1]; att = acc[:,0:64] * inv  (tensor_scalar)
  - att [t, hd] -> attT [hd, t] via SBUF->SBUF xbar DMA transpose (SP queue)
  - out-proj: lhsT = att^T (hd, tq), rhs = Wo (hd, c) -> out (tq, c)

Schedule: only chunk-0 projections run as a serial prefix; q/k/v projections
for chunks 1-3 and the out-projection are emitted as filler thunks inside
the ACT(exp)-paced attention loop so the PE pipeline never drains.
"""

import numpy as np
import ml_dtypes

B, T, C, H, D = 4, 2048, 1024, 16, 64
NCORES = 8
HPC = H // 2            # heads per core
HD = HPC * D            # 512, hidden per core
NPAIR = HPC // 2        # 4 head pairs
NCB = C // 128          # 8 contraction blocks
NTQ = T // 512          # 4 tq chunks
NTKB = T // 128         # 16 tk blocks
SCALE = float(1.0 / np.sqrt(np.float32(C)))
PIPE = 4                # QK->PV software pipeline depth

W_UP = 64.0             # host pre-scale on Wq/Wk: lifts 0.02-scale weights
#                         out of the fp8 subnormal range; exp() scale absorbs
#                         the 64*64 factor on the logits

BF16 = ml_dtypes.bfloat16
FP8 = ml_dtypes.float8_e4m3

_CACHE = {}


def build_program(repeat=1, stages='all'):
    """Build + compile the per-core Bass program (cached)."""
    key = ("nc", repeat, stages)
    if key in _CACHE:
        return _CACHE[key]

    import concourse.mybir as mybir
    import concourse.tile as tile
    from concourse import bacc
    from contextlib import ExitStack

    BF = mybir.dt.bfloat16
    F32 = mybir.dt.float32
    F8 = mybir.dt.float8e4

    nc = bacc.Bacc("TRN2", target_bir_lowering=False, debug=False,
                   enable_asserts=False, num_devices=NCORES)

    # q/k/v arrive pre-transposed from the host ([C, T]) so loads are plain
    # strided DMAs at full fan-out
    xq = nc.dram_tensor("xq", [C, T], F8, kind="ExternalInput").ap()
    xk = nc.dram_tensor("xk", [C, T], F8, kind="ExternalInput").ap()
    xv = nc.dram_tensor("xv", [C, T], BF, kind="ExternalInput").ap()
    wq = nc.dram_tensor("wq", [C, HD], F8, kind="ExternalInput").ap()
    wk = nc.dram_tensor("wk", [C, HD], F8, kind="ExternalInput").ap()
    wv = nc.dram_tensor("wv", [C, HD], BF, kind="ExternalInput").ap()
    wo = nc.dram_tensor("wo", [HD, C], BF, kind="ExternalInput").ap()
    out = nc.dram_tensor("out", [T, C], F32, kind="ExternalOutput").ap()

    EXP = mybir.ActivationFunctionType.Exp
    DR = mybir.MatmulPerfMode.DoubleRow
    ESC = SCALE / (W_UP * W_UP)

    with tile.TileContext(nc) as tc:
        with ExitStack() as ctx:
            if repeat > 1:
                ctx.enter_context(tc.For_i(0, repeat, 1))
            res = ctx.enter_context(tc.tile_pool(name="res", bufs=1))
            xTp = ctx.enter_context(tc.tile_pool(name="xT", bufs=6))
            ptp = ctx.enter_context(tc.tile_pool(name="pt", bufs=6))
            sbp = ctx.enter_context(tc.tile_pool(name="sb", bufs=6))
            stg = ctx.enter_context(tc.tile_pool(name="stg", bufs=4))
            outp = ctx.enter_context(tc.tile_pool(name="outS", bufs=2))
            mmA = tc.alloc_tile_pool(name="mmA", bufs=2, space="PSUM")

            # ---- input chunk loads (one strided DMA per tensor-chunk) ------
            xTc = {}  # (tensor, tchunk) -> [128, NCB*512] tile; cb-major cols

            def load_xT(key, src, j):
                dt = F8 if key in ("q", "k") else BF
                t = xTp.tile([128, NCB * 512], dt, tag="xT", name="xT")
                # t[p, cb*512 + i] = src[cb*128 + p, j*512 + i]
                nc.sync.dma_start(
                    t[:].rearrange("p (cb i) -> p cb i", i=512),
                    src[:, j * 512:(j + 1) * 512].rearrange(
                        "(cb p) i -> p cb i", p=128))
                xTc[(key, j)] = t

            def xT(key, j, cb):
                return xTc[(key, j)][:, cb * 512:(cb + 1) * 512]

            # ---- weights: each loads as ONE strided DMA into one wide tile -
            def load_w(dst, src_ap, nblk, width, eng):
                # dst[r, b*width+h] = src[b*128+r, h]
                eng.dma_start(
                    dst[:].rearrange("p (b h) -> p b h", h=width),
                    src_ap.rearrange("(b p) h -> p b h", p=128))

            wqall = res.tile([128, NCB * HD], F8, tag="wqall", name="wqall")
            load_w(wqall, wq, NCB, HD, nc.scalar)
            load_xT("q", xq, 0)
            wkall = res.tile([128, NCB * HD], F8, tag="wkall", name="wkall")
            load_w(wkall, wk, NCB, HD, nc.scalar)
            load_xT("k", xk, 0)
            wvall = res.tile([128, NCB * HD], BF, tag="wvall", name="wvall")
            load_w(wvall, wv, NCB, HD, nc.scalar)
            wv_sb = [wvall[:, cb * HD:(cb + 1) * HD] for cb in range(NCB)]
            load_xT("v", xv, 0)
            woall = res.tile([128, (HD // 128) * C], BF, tag="woall",
                             name="woall")
            load_w(woall, wo, HD // 128, C, nc.scalar)
            wo_sb = [woall[:, hb * C:(hb + 1) * C] for hb in range(HD // 128)]

            # ---- constants --------------------------------------------------
            maskT = res.tile([128, 128], BF, tag="maskT", name="maskT")
            nc.gpsimd.memset(maskT[:], 1.0)
            # maskT[i,j] = 0 where j < i (strictly-lower = future in [tk,tq])
            nc.gpsimd.affine_select(
                out=maskT[:], in_=maskT[:],
                compare_op=mybir.AluOpType.is_ge, fill=0.0,
                base=0, pattern=[[1, 128]], channel_multiplier=-1,
            )
            # sel65: one-hot row 64 — matmul lhsT that broadcasts the S row
            # of a pv tile to 64 PSUM partitions (custom-DVE recip only works
            # at partition offset 0, so S must be moved off partition 64)
            sel65 = res.tile([65, 64], BF, tag="sel65", name="sel65")
            nc.vector.memset(sel65[:], 0.0)
            nc.gpsimd.memset(sel65[64:65, :], 1.0)

            # ---- resident activations --------------------------------------
            # DoubleRow-layout projections: one tile per pair; head h on
            # partitions h*32..+32 (PE base partition must be 0/32/64),
            # chunk j at cols j*1024, d-half o at +o*512:
            #   qdr[p][h*32 + pp, j*1024 + o*512 + t'] = qh[d=o*32+pp, t]
            qdr = [res.tile([64, NTQ * 1024], F8, tag=f"qdr{p}",
                            name=f"qdr{p}") for p in range(NPAIR)]
            kdr = [res.tile([64, NTQ * 1024], F8, tag=f"kdr{p}",
                            name=f"kdr{p}") for p in range(NPAIR)]
            vha = [res.tile([128, HPC * 65], BF, tag=f"vha{kb}",
                            name=f"vha{kb}") for kb in range(NTKB)]
            attT = [res.tile([128, T], BF, tag=f"attT{p}", name=f"attT{p}")
                    for p in range(NPAIR)]
            # (attT rows 0:64 = head 2p, 64:128 = head 2p+1, cols = t)

            # ---- projection thunk builders ---------------------------------
            # wq/wk columns are HOST-permuted to col = p*128 + o*64 + h*32 +
            # pp (d = o*32 + pp), so each o-half matmul's 64 psum rows are
            # exactly the DR-tile partition layout -- the evac is a plain
            # partition-aligned DVE copy, no repack DMAs.
            def qk_thunks(key, wall, ddr, j, p, pool, tag):
                st = {}

                def mk_dr(o, cbp):
                    # fp8 DoubleRow: contraction pair = c-blocks (2cbp,
                    # 2cbp+1), interleaved via the middle AP dim (stride 512)
                    def f():
                        if cbp == 0:
                            st["ps"] = pool.tile([64, 512], F32, tag=tag,
                                                 name=tag)
                        lhsT = wall[:, 2 * cbp * HD:(2 * cbp + 2) * HD] \
                            .rearrange("pp (o2 m) -> pp o2 m", o2=2) \
                            [:, :, p * 128 + o * 64:p * 128 + (o + 1) * 64]
                        rhs = xTc[(key, j)] \
                            [:, 2 * cbp * 512:(2 * cbp + 2) * 512] \
                            .rearrange("pp (o2 i) -> pp o2 i", o2=2)
                        nc.tensor.matmul(
                            st["ps"][:], lhsT=lhsT, rhs=rhs,
                            perf_mode=DR,
                            start=(cbp == 0), stop=(cbp == NCB // 2 - 1))
                    return f

                def mk_evac(o):
                    def f():
                        nc.vector.tensor_copy(
                            ddr[p][0:64, j * 1024 + o * 512:
                                   j * 1024 + (o + 1) * 512],
                            st["ps"][:])
                    return f

                th = []
                for o in range(2):
                    th.extend(mk_dr(o, cbp) for cbp in range(NCB // 2))
                    th.append(mk_evac(o))
                return th

            def vproj_thunks(kb, pool, tag):
                st = {}

                def mk_mm(cb):
                    def f():
                        if cb == 0:
                            st["ps"] = pool.tile([128, 512], F32, tag=tag,
                                                 name=tag)
                        nc.tensor.matmul(
                            st["ps"][:],
                            lhsT=xT("v", kb // 4, cb)[:, (kb % 4) * 128:
                                                      (kb % 4 + 1) * 128],
                            rhs=wv_sb[cb][:],
                            start=(cb == 0), stop=(cb == NCB - 1))
                    return f

                def evac():
                    # one strided copy into the 65-stride layout + one strided
                    # memset of the 8 ones-columns
                    nc.vector.tensor_copy(
                        vha[kb][:].rearrange(
                            "p (h c) -> p h c", c=65)[:, :, 0:64],
                        st["ps"][:].rearrange("p (h c) -> p h c", c=64))
                    nc.vector.memset(
                        vha[kb][:].rearrange(
                            "p (h c) -> p h c", c=65)[:, :, 64:65], 1.0)

                return [mk_mm(cb) for cb in range(NCB)] + [evac]

            # ---- chunk-0 projections.  Only pair-0 q+k and v run as a
            # serial prefix so attention(0,0) unblocks ASAP; pairs 1-3 are
            # deferred to the filler queue (drain_for forces them just in
            # time for their attention).
            for f in qk_thunks("q", wqall, qdr, 0, 0, pool=mmA, tag="mmA"):
                f()
            for f in qk_thunks("k", wkall, kdr, 0, 0, pool=mmA, tag="mmA"):
                f()

            # ---- phase switch: release proj PSUM, open attention pools -----
            mmA.release()
            lgp = tc.alloc_tile_pool(name="lg", bufs=2, space="PSUM")
            accp = tc.alloc_tile_pool(name="acc", bufs=3, space="PSUM")
            mmp = tc.alloc_tile_pool(name="mm", bufs=1, space="PSUM")

            # ---- filler queues ---------------------------------------------
            normq = []           # norm thunks: highest priority
            vq = []              # (kb, thunk): v projections, drained on
            #                      demand by the PV that reads vha[kb]
            projq = []           # (chunk, need, thunk): q/k projections
            lowq = []            # out-projection thunks

            def pop_fillers(n=2):
                for _ in range(n):
                    if normq:
                        normq.pop(0)()
                    elif projq:
                        projq.pop(0)[2]()
                    elif vq:
                        vq.pop(0)[1]()
                    elif lowq:
                        lowq.pop(0)()
                    else:
                        return

            def drain_v(kb):
                """Force-emit v-proj thunks PV(kb) will read."""
                while vq and vq[0][0] <= kb:
                    vq.pop(0)[1]()

            def drain_for(j, p):
                """Force-emit proj thunks attention(p, j) will read
                (correctness: reads must be emitted after writes)."""
                while projq and (
                        projq[0][0] < j
                        or (projq[0][0] == j and projq[0][1] <= p + 1)):
                    projq.pop(0)[2]()

            # ---- attention --------------------------------------------------
            def attention(p, j, pops):
                ntk = 4 * (j + 1)
                c1 = (2 * p) * 65
                c2 = (2 * p + 1) * 65
                acc1 = accp.tile([65, 512], F32, tag="acc", name="acc")
                acc2 = accp.tile([65, 512], F32, tag="acc", name="acc")
                pend = []

                def emit_pv(it):
                    kb, o, w, pt = it
                    drain_v(kb)
                    nc.tensor.matmul(
                        acc1[:, o:512], lhsT=vha[kb][:, c1:c1 + 65],
                        rhs=pt[:, 0:w],
                        start=(kb == 0), stop=(kb == ntk - 1))
                    nc.tensor.matmul(
                        acc2[:, o:512], lhsT=vha[kb][:, c2:c2 + 65],
                        rhs=pt[:, 512:512 + w],
                        start=(kb == 0), stop=(kb == ntk - 1))

                for kb in range(ntk):
                    o = max(0, kb * 128 - j * 512)
                    w = 512 - o
                    kbj, kbr = kb // 4, kb % 4
                    lg = lgp.tile([128, 1024], F32, tag="lg", name="lg")
                    for h in range(2):
                        lhsT = kdr[p][h * 32:(h + 1) * 32,
                                      kbj * 1024:(kbj + 1) * 1024] \
                            .rearrange("p (o c) -> p o c", o=2) \
                            [:, :, kbr * 128:(kbr + 1) * 128]
                        rhs = qdr[p][h * 32:(h + 1) * 32,
                                     j * 1024:(j + 1) * 1024] \
                            .rearrange("p (o c) -> p o c", o=2) \
                            [:, :, o:o + w]
                        nc.tensor.matmul(
                            lg[:, h * 512:h * 512 + w], lhsT=lhsT, rhs=rhs,
                            perf_mode=DR, start=True, stop=True)
                    pt = ptp.tile([128, 1024], BF, tag="pt", name="pt")
                    # one exp covers both heads (3D AP on the diagonal)
                    if w == 512:
                        nc.scalar.activation(
                            pt[:, 0:1024], lg[:, 0:1024], EXP, scale=ESC)
                    else:
                        nc.scalar.activation(
                            pt[:].rearrange(
                                "p (h c) -> p h c", c=512)[:, :, 0:w],
                            lg[:].rearrange(
                                "p (h c) -> p h c", c=512)[:, :, 0:w],
                            EXP, scale=ESC)
                    if kb >= 4 * j:  # diagonal square: 0/1 mask at cols 0:128
                        nc.vector.tensor_mul(
                            pt[:, 0:128], pt[:, 0:128], maskT[:])
                        nc.vector.tensor_mul(
                            pt[:, 512:640], pt[:, 512:640], maskT[:])
                    pend.append((kb, o, w, pt))
                    pop_fillers(pops)
                    if len(pend) > PIPE:
                        emit_pv(pend.pop(0))
                for it in pend:
                    emit_pv(it)

                # normalize: att^T = pv^T * (1/S).  1/S on DVE (keeps every
                # ACT op an Exp so the table set loads once).  The custom
                # recip only reads partition 0, so S (psum row 64) is first
                # broadcast to 64 partitions with a one-hot matmul.
                for acc, row in ((acc1, 0), (acc2, 64)):
                    pv = sbp.tile([65, 512], BF, tag="pvE", name="pvE")
                    nc.vector.tensor_copy(pv[:], acc[:])

                    def norm_thunk(pv=pv, row=row, p=p, j=j):
                        sb_ps = accp.tile([64, 512], F32, tag="acc",
                                          name="acc")
                        nc.tensor.matmul(sb_ps[:], lhsT=sel65[:],
                                         rhs=pv[:], start=True, stop=True)
                        inv64 = sbp.tile([64, 512], F32, tag="inv",
                                         name="inv")
                        nc.vector.reciprocal_approx_fast(
                            inv64[:], sb_ps[:])
                        nc.vector.tensor_mul(
                            attT[p][row:row + 64,
                                    j * 512:(j + 1) * 512],
                            pv[0:64, :], inv64[:])
                    normq.append(norm_thunk)

            def outproj_thunks(tb, pool=None):
                pool = pool or mmp
                st = {}
                NHB = HD // 128

                def mk_mm(cc, hb):
                    def f():
                        if cc == 0 and hb == 0:
                            st["st"] = outp.tile([128, C], F32, tag="outS",
                                                 name="outS")
                        if hb == 0:
                            st["ps"] = pool.tile([128, 512], F32,
                                                 tag="mm", name="mm")
                        nc.tensor.matmul(
                            st["ps"][:],
                            lhsT=attT[hb][:, tb * 128:(tb + 1) * 128],
                            rhs=wo_sb[hb][:, cc * 512:(cc + 1) * 512],
                            start=(hb == 0), stop=(hb == NHB - 1))
                    return f

                def mk_evac(cc):
                    def f():
                        nc.vector.tensor_copy(
                            st["st"][:, cc * 512:(cc + 1) * 512], st["ps"][:])
                        if cc == C // 512 - 1:
                            nc.scalar.dma_start(
                                out[tb * 128:(tb + 1) * 128, :], st["st"][:])
                    return f

                th = []
                for cc in range(C // 512):
                    th.extend(mk_mm(cc, hb) for hb in range(NHB))
                    th.append(mk_evac(cc))
                return th

            # ---- main loop --------------------------------------------------
            POPS = [6, 5, 4, 3]
            for kb in range(4):
                vq.extend((kb, t) for t in vproj_thunks(kb, pool=mmp,
                                                        tag="mm"))
            for p in range(1, NPAIR):
                projq.extend(
                    (0, p + 1, t) for t in qk_thunks(
                        "q", wqall, qdr, 0, p, pool=mmp, tag="mm"))
                projq.extend(
                    (0, p + 1, t) for t in qk_thunks(
                        "k", wkall, kdr, 0, p, pool=mmp, tag="mm"))
            for j in range(NTQ):
                if j + 1 < NTQ:
                    load_xT("q", xq, j + 1)
                    load_xT("k", xk, j + 1)
                    load_xT("v", xv, j + 1)
                    for kb in range(4 * (j + 1), 4 * (j + 2)):
                        vq.extend(
                            (kb, t) for t in vproj_thunks(kb, pool=mmp,
                                                          tag="mm"))
                    for p in range(NPAIR):
                        projq.extend(
                            (j + 1, p + 1, t) for t in qk_thunks(
                                "q", wqall, qdr, j + 1, p, pool=mmp,
                                tag="mm"))
                        projq.extend(
                            (j + 1, p + 1, t) for t in qk_thunks(
                                "k", wkall, kdr, j + 1, p, pool=mmp,
                                tag="mm"))
                if j > 0:
                    for tb in range(4 * (j - 1), 4 * j):
                        lowq.extend(outproj_thunks(tb))
                for p in range(NPAIR):
                    drain_for(j, p)
                    attention(p, j, POPS[j])
            # drain in-loop fillers, then run the tail out-projection on a
            # wide PSUM pool carved from the released attention banks
            while normq or vq or projq or lowq:
                pop_fillers(8)
            mmp.release()
            accp.release()
            lgp.release()
            mmB = tc.alloc_tile_pool(name="mmB", bufs=4, space="PSUM")
            for tb in range(4 * (NTQ - 1), NTKB):
                for f in outproj_thunks(tb, pool=mmB):
                    f()
            mmB.release()

    nc.compile()
    _CACHE[key] = nc
    return nc


def make_in_maps(q, k, v, Wq, Wk, Wv, Wo):
    q = np.asarray(q, np.float32)
    k = np.asarray(k, np.float32)
    v = np.asarray(v, np.float32)
    Wq = np.asarray(Wq, np.float32)
    Wk = np.asarray(Wk, np.float32)
    Wv = np.asarray(Wv, np.float32)
    Wo = np.asarray(Wo, np.float32)

    def wslice(W, g, dt=BF16, scale=1.0):  # [H,C,D] -> [C, 8*D], group g
        return np.ascontiguousarray(
            W[g * HPC:(g + 1) * HPC].transpose(1, 0, 2).reshape(C, HD)
            * scale
        ).astype(dt)

    def wslice_dr(W, g, scale):
        # DoubleRow-friendly column order: col = p*128 + o*64 + h*32 + pp
        # <-> head (2p+h), d = o*32+pp
        Wp = W[g * HPC:(g + 1) * HPC] * scale          # [8, C, 64]
        arr = Wp.reshape(NPAIR, 2, C, 2, 32)           # [p, h, C, o, pp]
        arr = arr.transpose(2, 0, 3, 1, 4).reshape(C, HD)
        return np.ascontiguousarray(arr).astype(FP8)

    maps = []
    for core in range(NCORES):
        b, g = core // 2, core % 2
        maps.append({
            "xq": np.ascontiguousarray(q[b].T).astype(FP8),
            "xk": np.ascontiguousarray(k[b].T).astype(FP8),
            "xv": np.ascontiguousarray(v[b].T).astype(BF16),
            "wq": wslice_dr(Wq, g, W_UP),
            "wk": wslice_dr(Wk, g, W_UP),
            "wv": wslice(Wv, g),
            "wo": np.ascontiguousarray(Wo[g * HD:(g + 1) * HD]).astype(BF16),
        })
    return maps


def kernel(q, k, v, Wq, Wk, Wv, Wo, bo):
    from concourse.bass_utils import run_bass_kernel_spmd

    nc = build_program()
    in_maps = make_in_maps(q, k, v, Wq, Wk, Wv, Wo)
    res = run_bass_kernel_spmd(nc, in_maps, list(range(NCORES))).results
    bo = np.asarray(bo, np.float32)
    outv = np.empty((B, T, C), np.float32)
    for b in range(B):
        outv[b] = res[2 * b]["out"] + res[2 * b + 1]["out"]
    outv += bo
    return outv


# revision 35
# speedup vs baseline: 1.2361x; 1.2361x over previous
"""Bass/Tile TRN2 kernel for nn_MultiHead (B=4, T=2048, C=1024, H=16, D=64).

Sharding: 8 cores = batch(4) x head-group(2).  Each core computes, for its
batch b and its 8 heads, the full attention block and a *partial* output
projection (its 512 rows of Wo).  Host sums the two partials per batch and
adds the bias.

On-device layout trick: all activations are kept transposed
([feature, time]) so every matmul sees natural-layout operands:
  - q/k/v are DMA-transposed on load (bf16, xbar transpose), in 512-col chunks
  - q-proj / k-proj:  lhsT = Wq[cb]  (c,hd),  rhs = qT[cb] (c,t)  -> qh^T (hd,t)
  - v-proj:           lhsT = vT[cb]  (c,tk),  rhs = Wv[cb] (c,hd) -> vh (tk,hd)
  - QK^T:             lhsT = kh^T    (d,tk),  rhs = qh^T   (d,tq) -> logits^T (tk,tq)
    (two heads run concurrently via PE row-tiling: d=64 halves of the array;
     both land in one 2-bank PSUM tile so one ACT exp covers both heads)
  - softmax along partitions: ones-column appended to vh gives row sums for
    free inside the PV matmul; causal mask added before exp; exp on ACT
  - PV:               lhsT = vh_aug  (tk,65), rhs = P^T    (tk,tq) -> [pv^T; S] (65,tq)
  - normalize: 1/S on DVE (reciprocal_approx_fast, keeps ACT all-Exp so the
    activation table set loads exactly once), broadcast to 64 partitions via
    a K=1 matmul
  - out-proj:         lhsT = att^T   (hd,tq), rhs = Wo     (hd,c)  -> out (tq,c)

Schedule: only chunk-0 projections run as a serial prefix; q/k/v projections
for chunks 1-3 and the out-projection are emitted as filler thunks inside
the ACT(exp)-paced attention loop so the PE pipeline never drains.
"""

import numpy as np
import ml_dtypes

B, T, C, H, D = 4, 2048, 1024, 16, 64
NCORES = 8
HPC = H // 2            # heads per core
HD = HPC * D            # 512, hidden per core
NPAIR = HPC // 2        # 4 head pairs
NCB = C // 128          # 8 contraction blocks
NTQ = T // 512          # 4 tq chunks
NTKB = T // 128         # 16 tk blocks
SCALE = float(1.0 / np.sqrt(np.float32(C)))
PIPE = 3                # QK->PV software pipeline depth

FP8_QK = True           # fp8-e4m3 DoubleRow for the q/k projections
W_UP = 64.0             # host pre-scale on Wq/Wk: lifts 0.02-scale weights
#                         out of the fp8 subnormal range; exp() scale absorbs
#                         the 64*64 factor on the logits

BF16 = ml_dtypes.bfloat16
FP8 = ml_dtypes.float8_e4m3

_CACHE = {}


def build_program(repeat=1, stages='all'):
    """Build + compile the per-core Bass program (cached)."""
    key = ("nc", repeat, stages)
    if key in _CACHE:
        return _CACHE[key]

    import concourse.mybir as mybir
    import concourse.tile as tile
    from concourse import bacc
    from contextlib import ExitStack

    BF = mybir.dt.bfloat16
    F32 = mybir.dt.float32

    nc = bacc.Bacc("TRN2", target_bir_lowering=False, debug=False,
                   enable_asserts=False, num_devices=NCORES)

    F8 = mybir.dt.float8e4
    QK_DT = F8 if FP8_QK else BF
    # q/k/v arrive pre-transposed from the host ([C, T]) so loads are plain
    # strided DMAs at full fan-out — the xbar-transpose path serializes on a
    # single queue (~650ns per 128KB block)
    xq = nc.dram_tensor("xq", [C, T], QK_DT, kind="ExternalInput").ap()
    xk = nc.dram_tensor("xk", [C, T], QK_DT, kind="ExternalInput").ap()
    xv = nc.dram_tensor("xv", [C, T], BF, kind="ExternalInput").ap()
    wq = nc.dram_tensor("wq", [C, HD], QK_DT, kind="ExternalInput").ap()
    wk = nc.dram_tensor("wk", [C, HD], QK_DT, kind="ExternalInput").ap()
    wv = nc.dram_tensor("wv", [C, HD], BF, kind="ExternalInput").ap()
    wo = nc.dram_tensor("wo", [HD, C], BF, kind="ExternalInput").ap()
    out = nc.dram_tensor("out", [T, C], F32, kind="ExternalOutput").ap()

    EXP = mybir.ActivationFunctionType.Exp
    ESC = SCALE / (W_UP * W_UP) if FP8_QK else SCALE

    with tile.TileContext(nc) as tc:
        with ExitStack() as ctx:
            if repeat > 1:
                ctx.enter_context(tc.For_i(0, repeat, 1))
            res = ctx.enter_context(tc.tile_pool(name="res", bufs=1))
            xTp = ctx.enter_context(tc.tile_pool(name="xT", bufs=6))
            ptp = ctx.enter_context(tc.tile_pool(name="pt", bufs=4))
            sbp = ctx.enter_context(tc.tile_pool(name="sb", bufs=6))
            outp = ctx.enter_context(tc.tile_pool(name="outS", bufs=2))
            mmA = tc.alloc_tile_pool(name="mmA", bufs=2, space="PSUM")

            # ---- input chunk loads (one strided DMA per tensor-chunk) ------
            xTc = {}  # (tensor, tchunk) -> [128, NCB*512] tile; cb-major cols

            def load_xT(key, src, j):
                dt = QK_DT if key in ("q", "k") else BF
                t = xTp.tile([128, NCB * 512], dt, tag="xT", name="xT")
                # t[p, cb*512 + i] = src[cb*128 + p, j*512 + i]
                nc.sync.dma_start(
                    t[:].rearrange("p (cb i) -> p cb i", i=512),
                    src[:, j * 512:(j + 1) * 512].rearrange(
                        "(cb p) i -> p cb i", p=128))
                xTc[(key, j)] = t

            def xT(key, j, cb):
                return xTc[(key, j)][:, cb * 512:(cb + 1) * 512]

            # ---- weights: each loads as ONE strided DMA into one wide tile -
            def load_w(dst, src_ap, nblk, width, eng):
                # dst[r, b*width+h] = src[b*128+r, h]
                eng.dma_start(
                    dst[:].rearrange("p (b h) -> p b h", h=width),
                    src_ap.rearrange("(b p) h -> p b h", p=128))

            wqall = res.tile([128, NCB * HD], QK_DT, tag="wqall",
                             name="wqall")
            load_w(wqall, wq, NCB, HD, nc.scalar)
            wq_sb = [wqall[:, cb * HD:(cb + 1) * HD] for cb in range(NCB)]
            load_xT("q", xq, 0)
            wkall = res.tile([128, NCB * HD], QK_DT, tag="wkall",
                             name="wkall")
            load_w(wkall, wk, NCB, HD, nc.scalar)
            wk_sb = [wkall[:, cb * HD:(cb + 1) * HD] for cb in range(NCB)]
            load_xT("k", xk, 0)
            wvall = res.tile([128, NCB * HD], BF, tag="wvall", name="wvall")
            load_w(wvall, wv, NCB, HD, nc.scalar)
            wv_sb = [wvall[:, cb * HD:(cb + 1) * HD] for cb in range(NCB)]
            load_xT("v", xv, 0)
            woall = res.tile([128, (HD // 128) * C], BF, tag="woall",
                             name="woall")
            load_w(woall, wo, HD // 128, C, nc.scalar)
            wo_sb = [woall[:, hb * C:(hb + 1) * C] for hb in range(HD // 128)]

            # ---- constants --------------------------------------------------
            maskT = res.tile([128, 128], BF, tag="maskT", name="maskT")
            nc.gpsimd.memset(maskT[:], 1.0)
            # maskT[i,j] = 0 where j < i (strictly-lower = future in [tk,tq])
            nc.gpsimd.affine_select(
                out=maskT[:], in_=maskT[:],
                compare_op=mybir.AluOpType.is_ge, fill=0.0,
                base=0, pattern=[[1, 128]], channel_multiplier=-1,
            )

            # sel65: one-hot row 64 — matmul lhsT that broadcasts the S row
            # of a pv tile to 64 PSUM partitions (custom-DVE recip only works
            # at partition offset 0, so S must be moved off partition 64)
            sel65 = res.tile([65, 64], BF, tag="sel65", name="sel65")
            nc.vector.memset(sel65[:], 0.0)
            nc.gpsimd.memset(sel65[64:65, :], 1.0)

            # ---- resident activations --------------------------------------
            qhT = [res.tile([128, T], BF, tag=f"qhT{p}", name=f"qhT{p}")
                   for p in range(NPAIR)]
            khT = [res.tile([128, T], BF, tag=f"khT{p}", name=f"khT{p}")
                   for p in range(NPAIR)]
            vha = [res.tile([128, HPC * 65], BF, tag=f"vha{kb}", name=f"vha{kb}")
                   for kb in range(NTKB)]
            attT = [res.tile([128, T], BF, tag=f"attT{p}", name=f"attT{p}")
                    for p in range(NPAIR)]

            # ---- projection thunk builders ---------------------------------
            def qk_thunks(key, wall, dstT, j, p, pool, tag):
                st = {}
                w_sb = wq_sb if wall is wqall else wk_sb

                def mk(cb):
                    def f():
                        if cb == 0:
                            st["ps"] = pool.tile([128, 512], F32, tag=tag,
                                                 name=tag)
                        nc.tensor.matmul(
                            st["ps"][:],
                            lhsT=w_sb[cb][:, p * 128:(p + 1) * 128],
                            rhs=xT(key, j, cb),
                            start=(cb == 0), stop=(cb == NCB - 1))
                    return f

                def mk_dr(cbp):
                    # fp8 DoubleRow: contraction pair = c-blocks (2cbp,
                    # 2cbp+1), interleaved via the middle AP dim (stride 512)
                    def f():
                        if cbp == 0:
                            st["ps"] = pool.tile([128, 512], F32, tag=tag,
                                                 name=tag)
                        lhsT = wall[:, 2 * cbp * HD:(2 * cbp + 2) * HD] \
                            .rearrange("pp (o m) -> pp o m", o=2) \
                            [:, :, p * 128:(p + 1) * 128]
                        rhs = xTc[(key, j)] \
                            [:, 2 * cbp * 512:(2 * cbp + 2) * 512] \
                            .rearrange("pp (o i) -> pp o i", o=2)
                        nc.tensor.matmul(
                            st["ps"][:], lhsT=lhsT, rhs=rhs,
                            perf_mode=mybir.MatmulPerfMode.DoubleRow,
                            start=(cbp == 0), stop=(cbp == NCB // 2 - 1))
                    return f

                def evac():
                    nc.vector.tensor_copy(
                        dstT[p][:, j * 512:(j + 1) * 512], st["ps"][:])

                if FP8_QK:
                    return [mk_dr(cbp) for cbp in range(NCB // 2)] + [evac]
                return [mk(cb) for cb in range(NCB)] + [evac]

            def vproj_thunks(kb, pool, tag):
                st = {}

                def mk_mm(cb):
                    def f():
                        if cb == 0:
                            st["ps"] = pool.tile([128, 512], F32, tag=tag,
                                                 name=tag)
                        nc.tensor.matmul(
                            st["ps"][:],
                            lhsT=xT("v", kb // 4, cb)[:, (kb % 4) * 128:
                                                      (kb % 4 + 1) * 128],
                            rhs=wv_sb[cb][:],
                            start=(cb == 0), stop=(cb == NCB - 1))
                    return f

                def evac():
                    # one strided copy into the 65-stride layout + one strided
                    # memset of the 8 ones-columns (vs 16 narrow DVE ops)
                    nc.vector.tensor_copy(
                        vha[kb][:].rearrange(
                            "p (h c) -> p h c", c=65)[:, :, 0:64],
                        st["ps"][:].rearrange("p (h c) -> p h c", c=64))
                    nc.vector.memset(
                        vha[kb][:].rearrange(
                            "p (h c) -> p h c", c=65)[:, :, 64:65], 1.0)

                return [mk_mm(cb) for cb in range(NCB)] + [evac]

            # ---- chunk-0 projections (serial prefix, double-buffered PSUM).
            # q/k first: their inputs are the first items on both DMA rings,
            # so PE starts ~6us in instead of waiting for v/wv.
            for p in range(NPAIR):
                for f in qk_thunks("q", wqall, qhT, 0, p, pool=mmA,
                                   tag="mmA"):
                    f()
            for p in range(NPAIR):
                for f in qk_thunks("k", wkall, khT, 0, p, pool=mmA,
                                   tag="mmA"):
                    f()
            for kb in range(4):
                for f in vproj_thunks(kb, pool=mmA, tag="mmA"):
                    f()

            # ---- phase switch: release proj PSUM, open attention pools -----
            mmA.release()
            lgp = tc.alloc_tile_pool(name="lg", bufs=2, space="PSUM")
            accp = tc.alloc_tile_pool(name="acc", bufs=3, space="PSUM")
            mmp = tc.alloc_tile_pool(name="mm", bufs=1, space="PSUM")

            # ---- filler queues ---------------------------------------------
            normq = []           # norm thunks: highest priority
            projq = []           # (chunk, need, thunk): q/k/v projections;
            #                      need = 0 for v (any pair), p+1 for pair p
            lowq = []            # out-projection thunks

            def pop_fillers(n=2):
                for _ in range(n):
                    if normq:
                        normq.pop(0)()
                    elif projq:
                        projq.pop(0)[2]()
                    elif lowq:
                        lowq.pop(0)()
                    else:
                        return

            def drain_for(j, p):
                """Force-emit proj thunks attention(p, j) will read
                (correctness: reads must be emitted after writes)."""
                while projq and (
                        projq[0][0] < j
                        or (projq[0][0] == j and projq[0][1] <= p + 1)):
                    projq.pop(0)[2]()

            # ---- attention --------------------------------------------------
            def attention(p, j, pops):
                c1 = (2 * p) * 65
                c2 = (2 * p + 1) * 65
                ntk = 4 * (j + 1)
                acc1 = accp.tile([65, 512], F32, tag="acc", name="acc")
                acc2 = accp.tile([65, 512], F32, tag="acc", name="acc")
                pend = []

                def emit_pv(it):
                    kb, o, w, pt = it
                    nc.tensor.matmul(
                        acc1[:, o:512], lhsT=vha[kb][:, c1:c1 + 65],
                        rhs=pt[:, 0:w],
                        start=(kb == 0), stop=(kb == ntk - 1))
                    nc.tensor.matmul(
                        acc2[:, o:512], lhsT=vha[kb][:, c2:c2 + 65],
                        rhs=pt[:, 512:512 + w],
                        start=(kb == 0), stop=(kb == ntk - 1))

                for kb in range(ntk):
                    o = max(0, kb * 128 - j * 512)
                    w = 512 - o
                    lg = lgp.tile([128, 1024], F32, tag="lg", name="lg")
                    nc.tensor.matmul(
                        lg[:, 0:w],
                        lhsT=khT[p][0:64, kb * 128:(kb + 1) * 128],
                        rhs=qhT[p][0:64, j * 512 + o:(j + 1) * 512],
                        start=True, stop=True)
                    nc.tensor.matmul(
                        lg[:, 512:512 + w],
                        lhsT=khT[p][64:128, kb * 128:(kb + 1) * 128],
                        rhs=qhT[p][64:128, j * 512 + o:(j + 1) * 512],
                        start=True, stop=True)
                    pt = ptp.tile([128, 1024], BF, tag="pt", name="pt")
                    if kb >= 4 * j:  # diagonal block: 2 narrow exps + 0/1 mask
                        nc.scalar.activation(
                            pt[:, 0:w], lg[:, 0:w], EXP, scale=ESC)
                        nc.scalar.activation(
                            pt[:, 512:512 + w], lg[:, 512:512 + w], EXP,
                            scale=ESC)
                        nc.vector.tensor_mul(
                            pt[:, 0:128], pt[:, 0:128], maskT[:])
                        nc.vector.tensor_mul(
                            pt[:, 512:640], pt[:, 512:640], maskT[:])
                    else:            # one exp covering both heads
                        nc.scalar.activation(
                            pt[:, 0:1024], lg[:, 0:1024], EXP, scale=ESC)
                    pend.append((kb, o, w, pt))
                    pop_fillers(pops)
                    if len(pend) > PIPE:
                        emit_pv(pend.pop(0))
                for it in pend:
                    emit_pv(it)

                # normalize: att^T = pv^T * (1/S).  1/S runs on DVE (keeps
                # every ACT op an Exp -> the activation table set loads once;
                # Ln forced a ~2.7us set switch per normalize on HW).  The
                # custom recip only reads partition 0, so S (psum row 64) is
                # first broadcast to 64 partitions with a one-hot matmul.
                for acc, row in ((acc1, 0), (acc2, 64)):
                    pv = sbp.tile([65, 512], BF, tag="pvE", name="pvE")
                    nc.vector.tensor_copy(pv[:], acc[:])

                    def norm_thunk(pv=pv, row=row, p=p, j=j):
                        sb_ps = accp.tile([64, 512], F32, tag="acc",
                                          name="acc")
                        nc.tensor.matmul(sb_ps[:], lhsT=sel65[:],
                                         rhs=pv[:], start=True, stop=True)
                        inv64 = sbp.tile([64, 512], F32, tag="inv",
                                         name="inv")
                        nc.vector.reciprocal_approx_fast(
                            inv64[:], sb_ps[:])
                        nc.vector.tensor_mul(
                            attT[p][row:row + 64,
                                    j * 512:(j + 1) * 512],
                            pv[0:64, :], inv64[:])
                    normq.insert(0, norm_thunk)

            def outproj_thunks(tb, pool=None):
                pool = pool or mmp
                st = {}
                NHB = HD // 128

                def mk_mm(cc, hb):
                    def f():
                        if cc == 0 and hb == 0:
                            st["st"] = outp.tile([128, C], F32, tag="outS",
                                                 name="outS")
                        if hb == 0:
                            st["ps"] = pool.tile([128, 512], F32,
                                                 tag="mm", name="mm")
                        nc.tensor.matmul(
                            st["ps"][:],
                            lhsT=attT[hb][:, tb * 128:(tb + 1) * 128],
                            rhs=wo_sb[hb][:, cc * 512:(cc + 1) * 512],
                            start=(hb == 0), stop=(hb == NHB - 1))
                    return f

                def mk_evac(cc):
                    def f():
                        nc.vector.tensor_copy(
                            st["st"][:, cc * 512:(cc + 1) * 512], st["ps"][:])
                        if cc == C // 512 - 1:
                            nc.scalar.dma_start(
                                out[tb * 128:(tb + 1) * 128, :], st["st"][:])
                    return f

                th = []
                for cc in range(C // 512):
                    th.extend(mk_mm(cc, hb) for hb in range(NHB))
                    th.append(mk_evac(cc))
                return th

            # ---- main loop --------------------------------------------------
            POPS = [4, 3, 2, 2]
            for j in range(NTQ):
                if j + 1 < NTQ:
                    load_xT("q", xq, j + 1)
                    load_xT("k", xk, j + 1)
                    load_xT("v", xv, j + 1)
                    for kb in range(4 * (j + 1), 4 * (j + 2)):
                        projq.extend(
                            (j + 1, 0, t) for t in vproj_thunks(kb, pool=mmp,
                                                                tag="mm"))
                    for p in range(NPAIR):
                        projq.extend(
                            (j + 1, p + 1, t) for t in qk_thunks(
                                "q", wqall, qhT, j + 1, p, pool=mmp,
                                tag="mm"))
                        projq.extend(
                            (j + 1, p + 1, t) for t in qk_thunks(
                                "k", wkall, khT, j + 1, p, pool=mmp,
                                tag="mm"))
                if j > 0:
                    for tb in range(4 * (j - 1), 4 * j):
                        lowq.extend(outproj_thunks(tb))
                for p in range(NPAIR):
                    drain_for(j, p)
                    attention(p, j, POPS[j])
            # drain in-loop fillers, then run the tail out-projection on a
            # wide PSUM pool carved from the released attention banks
            while normq or projq or lowq:
                pop_fillers(8)
            mmp.release()
            accp.release()
            lgp.release()
            mmB = tc.alloc_tile_pool(name="mmB", bufs=4, space="PSUM")
            for tb in range(4 * (NTQ - 1), NTKB):
                for f in outproj_thunks(tb, pool=mmB):
                    f()
            mmB.release()

    nc.compile()
    _CACHE[key] = nc
    return nc


def make_in_maps(q, k, v, Wq, Wk, Wv, Wo):
    q = np.asarray(q, np.float32)
    k = np.asarray(k, np.float32)
    v = np.asarray(v, np.float32)
    Wq = np.asarray(Wq, np.float32)
    Wk = np.asarray(Wk, np.float32)
    Wv = np.asarray(Wv, np.float32)
    Wo = np.asarray(Wo, np.float32)

    def wslice(W, g, dt=BF16, scale=1.0):  # [H,C,D] -> [C, 8*D], group g
        return np.ascontiguousarray(
            W[g * HPC:(g + 1) * HPC].transpose(1, 0, 2).reshape(C, HD)
            * scale
        ).astype(dt)

    qk_dt = FP8 if FP8_QK else BF16
    w_up = W_UP if FP8_QK else 1.0
    maps = []
    for core in range(NCORES):
        b, g = core // 2, core % 2
        maps.append({
            "xq": np.ascontiguousarray(q[b].T).astype(qk_dt),
            "xk": np.ascontiguousarray(k[b].T).astype(qk_dt),
            "xv": np.ascontiguousarray(v[b].T).astype(BF16),
            "wq": wslice(Wq, g, qk_dt, w_up),
            "wk": wslice(Wk, g, qk_dt, w_up),
            "wv": wslice(Wv, g),
            "wo": np.ascontiguousarray(Wo[g * HD:(g + 1) * HD]).astype(BF16),
        })
    return maps


def kernel(q, k, v, Wq, Wk, Wv, Wo, bo):
    from concourse.bass_utils import run_bass_kernel_spmd

    nc = build_program()
    in_maps = make_in_maps(q, k, v, Wq, Wk, Wv, Wo)
    res = run_bass_kernel_spmd(nc, in_maps, list(range(NCORES))).results
    bo = np.asarray(bo, np.float32)
    outv = np.empty((B, T, C), np.float32)
    for b in range(B):
        outv[b] = res[2 * b]["out"] + res[2 * b + 1]["out"]
    outv += bo
    return outv

